# revision 1
# baseline (speedup 1.0000x reference)
"""Edge-MLP GNN message passing kernel for Trainium2 (8 NeuronCores).

Computes, for each edge e = (u, v):
    out[e] = sigmoid(relu(|x[u] - x[v]| @ W1 + b1) @ W2 + b2)

Strategy (data parallel over edges, x + weights replicated):
  - Host casts x to fp16; it stays in HBM as the gather source.
  - Per core (80000 edges): dma_gather(transpose=True) fetches both
    endpoints' feature rows as [128 feat, C edges] columns in SBUF
    (the DMA XBar does the transpose for free).
  - DVE: d = g0 - g1 (2x mode), |d| via abs_max(d, 0) (4x mode).
  - PE: h = W1.T @ d, two 512-edge groups packed into PSUM partition
    halves (tile_position col offset 64) for full-width epilogues.
  - DVE: relu(h + b1) fused via tensor_scalar(add, max), cast to fp16.
  - PE: per 128-edge group, matmul(lhsT=h[64,128], rhs=W2[64,1]) ->
    [128, 1] PSUM column => edges land on partitions.
  - ACT: sigmoid(psum + b2) -> out_sb; single DMA to DRAM at the end.
  - Host reassembles: out[p, c] = edge c*128 + p per core.
"""

import os
import sys

for _p in ("/opt/trn_rl_repo", "/root/.axon_site/_ro/trn_rl_repo"):
    if os.path.isdir(_p) and _p not in sys.path:
        sys.path.insert(0, _p)

import numpy as np

import concourse.bacc as bacc
import concourse.mybir as mybir
from concourse.mybir import AluOpType
from concourse.tile import TileContext
from concourse.bass_utils import run_bass_kernel_spmd

N_NODES = 10000
N_EDGES = 640000
D_FEAT = 128
HID = 64
N_CORES = 8
E_CORE = N_EDGES // N_CORES  # 80000 edges per core

CHUNK = 8192  # edges per gather chunk (must be % 128)

f16 = mybir.dt.float16
f32 = mybir.dt.float32
i16 = mybir.dt.int16

_NC_CACHE = None


def _chunk_list(e_core, chunk):
    chunks = []
    rem = e_core
    while rem > 0:
        c = min(chunk, rem)
        chunks.append(c)
        rem -= c
    return chunks


def _build_nc(e_core=E_CORE, chunk=CHUNK):
    """Build + finalize the (SPMD, per-core identical) Bass kernel."""
    _chunks = _chunk_list(e_core, chunk)
    n_out_cols = e_core // 128

    nc = bacc.Bacc("TRN2", target_bir_lowering=False)

    x16 = nc.dram_tensor("x16", [N_NODES, D_FEAT], f16, kind="ExternalInput")
    idx0_d = nc.dram_tensor("idx0", [128, e_core // 16], i16, kind="ExternalInput")
    idx1_d = nc.dram_tensor("idx1", [128, e_core // 16], i16, kind="ExternalInput")
    w1_d = nc.dram_tensor("w1", [D_FEAT, HID], f16, kind="ExternalInput")
    w2_d = nc.dram_tensor("w2", [128, 1], f16, kind="ExternalInput")  # W2 stacked 2x
    b1_d = nc.dram_tensor("b1", [128, 1], f32, kind="ExternalInput")  # b1 stacked 2x
    b2_d = nc.dram_tensor("b2", [128, 1], f32, kind="ExternalInput")  # b2 bcast
    out_d = nc.dram_tensor("out", [128, n_out_cols], f32, kind="ExternalOutput")

    with TileContext(nc) as tc:
        with (
            tc.tile_pool(name="const", bufs=1) as cpool,
            tc.tile_pool(name="gather", bufs=3) as gpool,
            tc.tile_pool(name="diff", bufs=2) as dpool,
            tc.tile_pool(name="hid", bufs=4) as hpool,
            tc.tile_pool(name="outp", bufs=1) as opool,
            tc.tile_pool(name="ps1", bufs=4, space="PSUM") as ppool,
            tc.tile_pool(name="ps2", bufs=2, space="PSUM") as p2pool,
        ):
            idx0 = cpool.tile([128, e_core // 16], i16, tag="idx0")
            idx1 = cpool.tile([128, e_core // 16], i16, tag="idx1")
            w1 = cpool.tile([D_FEAT, HID], f16, tag="w1")
            w2 = cpool.tile([128, 1], f16, tag="w2")
            b1 = cpool.tile([128, 1], f32, tag="b1")
            b2 = cpool.tile([128, 1], f32, tag="b2")
            out_sb = opool.tile([128, n_out_cols], f32, tag="osb")

            nc.sync.dma_start(idx0[:], idx0_d[:])
            nc.sync.dma_start(idx1[:], idx1_d[:])
            nc.sync.dma_start(w1[:], w1_d[:])
            nc.sync.dma_start(w2[:], w2_d[:])
            nc.sync.dma_start(b1[:], b1_d[:])
            nc.sync.dma_start(b2[:], b2_d[:])

            e0 = 0  # first edge of chunk
            col0 = 0  # first out_sb column of chunk
            for C in _chunks:
                g0 = gpool.tile([128, C], f16, tag="g0")
                g1 = gpool.tile([128, C], f16, tag="g1")
                c0 = e0 // 16
                nc.gpsimd.dma_gather(
                    g0[:].rearrange("p (a c) -> p a c", a=1),
                    x16[:],
                    idx0[:, c0 : c0 + C // 16],
                    C,
                    C,
                    elem_size=D_FEAT,
                    transpose=True,
                    single_packet=False,
                )
                nc.gpsimd.dma_gather(
                    g1[:].rearrange("p (a c) -> p a c", a=1),
                    x16[:],
                    idx1[:, c0 : c0 + C // 16],
                    C,
                    C,
                    elem_size=D_FEAT,
                    transpose=True,
                    single_packet=False,
                )
                d = dpool.tile([128, C], f16, tag="d")
                nc.vector.tensor_tensor(d[:], g0[:], g1[:], AluOpType.subtract)
                # |d| on the (otherwise idle) scalar engine
                nc.scalar.activation(
                    d[:], d[:], mybir.ActivationFunctionType.Abs,
                )

                ncols = C // 128
                p2 = p2pool.tile([128, ncols], f32, tag="p2")
                colc = 0
                for g in range(0, C, 1024):
                    nA = min(512, C - g)
                    nB = min(512, C - g - nA)
                    pm = ppool.tile([128, 512], f32, tag="pm")
                    nc.tensor.matmul(
                        pm[0:HID, 0:nA], w1[:], d[:, g : g + nA],
                        start=True, stop=True,
                    )
                    if nB:
                        nc.tensor.matmul(
                            pm[HID:128, 0:nB], w1[:], d[:, g + nA : g + nA + nB],
                            start=True, stop=True,
                        )
                    h = hpool.tile([128, 512], f16, tag="h")
                    if nB == nA:
                        nc.vector.tensor_scalar(
                            h[:, 0:nA], pm[:, 0:nA], b1[:], 0.0,
                            AluOpType.add, AluOpType.max,
                        )
                    else:
                        nc.vector.tensor_scalar(
                            h[0:HID, 0:nA], pm[0:HID, 0:nA], b1[0:HID, :], 0.0,
                            AluOpType.add, AluOpType.max,
                        )
                        if nB:
                            nc.vector.tensor_scalar(
                                h[HID:128, 0:nB], pm[HID:128, 0:nB], b1[HID:128, :],
                                0.0, AluOpType.add, AluOpType.max,
                            )
                    for j in range(nA // 128):
                        nc.tensor.matmul(
                            p2[:, colc : colc + 1],
                            h[0:HID, j * 128 : (j + 1) * 128],
                            w2[0:HID, :],
                            start=True, stop=True,
                        )
                        colc += 1
                    for j in range(nB // 128):
                        nc.tensor.matmul(
                            p2[:, colc : colc + 1],
                            h[HID:128, j * 128 : (j + 1) * 128],
                            w2[HID:128, :],
                            start=True, stop=True,
                        )
                        colc += 1
                nc.scalar.activation(
                    out_sb[:, col0 : col0 + ncols], p2[:, 0:ncols],
                    mybir.ActivationFunctionType.Sigmoid,
                    bias=b2[:], scale=1.0,
                )
                e0 += C
                col0 += ncols

            nc.sync.dma_start(out_d[:], out_sb[:])

    nc.finalize()
    return nc


def _get_nc():
    global _NC_CACHE
    if _NC_CACHE is None:
        _NC_CACHE = _build_nc()
    return _NC_CACHE


def _interleave_idx(a):
    """[e_core] int array -> [128, e_core//16] int16 SWDGE index layout.

    dma_gather consumes index i from [i % 16, i // 16]; rows are wrapped in
    16 partitions and replicated 8x for the 8 Q7 cores.
    """
    e_core = a.shape[0]
    m = a.reshape(e_core // 16, 16).T.astype(np.int16)  # [16, E/16]
    return np.tile(m, (8, 1))  # [128, E/16]


def prep_in_maps(x, indices, W1, b1, W2, b2):
    x16 = np.ascontiguousarray(np.asarray(x, dtype=np.float32)).astype(np.float16)
    idx = np.asarray(indices)
    w1 = np.asarray(W1, dtype=np.float32).astype(np.float16)
    w2c = np.asarray(W2, dtype=np.float32).astype(np.float16).reshape(HID, 1)
    w2s = np.concatenate([w2c, w2c], axis=0)  # [128, 1]
    b1c = np.asarray(b1, dtype=np.float32).reshape(HID, 1)
    b1s = np.concatenate([b1c, b1c], axis=0)  # [128, 1]
    b2s = np.full((128, 1), np.asarray(b2, dtype=np.float32).reshape(-1)[0],
                  dtype=np.float32)

    in_maps = []
    for c in range(N_CORES):
        sl = slice(c * E_CORE, (c + 1) * E_CORE)
        in_maps.append({
            "x16": x16,
            "idx0": _interleave_idx(idx[0, sl]),
            "idx1": _interleave_idx(idx[1, sl]),
            "w1": w1,
            "w2": w2s,
            "b1": b1s,
            "b2": b2s,
        })
    return in_maps


def run_hw(x, indices, W1, b1, W2, b2, trace=False, **kw):
    """Run on the 8 NeuronCores; returns (out [N_EDGES] f32, BassKernelResults)."""
    nc = _get_nc()
    in_maps = prep_in_maps(x, indices, W1, b1, W2, b2)
    res = run_bass_kernel_spmd(
        nc, in_maps, core_ids=list(range(N_CORES)), trace=trace, **kw
    )
    outs = []
    for c in range(N_CORES):
        o = np.asarray(res.results[c]["out"])  # [128, 625]
        outs.append(o.T.reshape(-1))  # edge e = col*128 + p
    return np.concatenate(outs), res


def kernel(x, indices, W1, b1, W2, b2):
    out, _ = run_hw(x, indices, W1, b1, W2, b2, trace=False)
    return out.astype(np.float32)



# revision 2
# speedup vs baseline: 1.1047x; 1.1047x over previous
"""Edge-MLP GNN message passing kernel for Trainium2 (8 NeuronCores), v4.

out[e] = sigmoid(relu(|x[u_e] - x[v_e]| @ W1 + b1) @ W2 + b2)

v4 splits the two gathers across independent engines:
  - Edges are sorted by u per core and binned into 40 windows of 256 node
    ids, each padded to a fixed 2176 edges (17 blocks of 128) so the window
    schedule is compile-time static and identical across cores (SPMD).
  - u-side gather runs on the TENSOR engine: one-hot columns are built with
    tensor_scalar(is_equal) against per-partition iota scalars, then
    x_win.T @ onehot accumulates x[u] columns into PSUM (two matmuls per
    block: window halves w*256 and w*256+128). No SWDGE descriptors at all.
  - v-side gather stays on SWDGE dma_gather (the Q7 descriptor generator is
    the machine's only fast arbitrary-index path and binds at ~8.5ns/idx —
    halving its work is the whole point of v4).
  - DVE: d = |psum_u - g_v| via subtract + abs_max.
  - PE mm1 packed halves, ACT relu, PE mm2 via w2two pairing, ACT sigmoid.
  - Host pre-sorts/pads per core and inverts the permutation at the end.
"""

import os
import sys

for _p in ("/opt/trn_rl_repo", "/root/.axon_site/_ro/trn_rl_repo"):
    if os.path.isdir(_p) and _p not in sys.path:
        sys.path.insert(0, _p)

import numpy as np

import concourse.bacc as bacc
import concourse.mybir as mybir
from concourse.mybir import AluOpType
from concourse.tile import TileContext
from concourse.bass_utils import run_bass_kernel_spmd

N_NODES = 10000
N_EDGES = 640000
D_FEAT = 128
HID = 64
N_CORES = 8
E_CORE = N_EDGES // N_CORES      # 80000

W_WIN = 256                      # node ids per window
WINDOWS = 40                     # 40 * 256 = 10240 >= 10000
EPW = 2304                       # padded edges per window (18 blocks of 128)
BPW = EPW // 128                 # 18
E_PAD = WINDOWS * EPW            # 92160
COLS = E_PAD // 128              # 720
N_PADNODES = WINDOWS * W_WIN     # 10240

f16 = mybir.dt.float16
f32 = mybir.dt.float32
i16 = mybir.dt.int16

_NC_CACHE = None


def _build_nc():
    nc = bacc.Bacc("TRN2", target_bir_lowering=False)

    x16_d = nc.dram_tensor("x16", [N_NODES, D_FEAT], f16, kind="ExternalInput")
    # node-major pack: xpack[p, blk*128 + f] = x[blk*128 + p, f]
    xpack_d = nc.dram_tensor("xpack", [128, N_PADNODES], f16,
                             kind="ExternalInput")
    idxv_d = nc.dram_tensor("idxv", [128, E_PAD // 16], i16,
                            kind="ExternalInput")
    urel_d = nc.dram_tensor("urel", [1, E_PAD], f16, kind="ExternalInput")
    iota_d = nc.dram_tensor("iota2", [128, 2], f16, kind="ExternalInput")
    w1_d = nc.dram_tensor("w1", [D_FEAT, HID], f16, kind="ExternalInput")
    w2two_d = nc.dram_tensor("w2two", [128, 2], f16, kind="ExternalInput")
    b1_d = nc.dram_tensor("b1", [128, 1], f32, kind="ExternalInput")
    b2_d = nc.dram_tensor("b2", [128, 1], f32, kind="ExternalInput")
    out_d = nc.dram_tensor("out", [128, COLS], f32, kind="ExternalOutput")

    with TileContext(nc) as tc:
        with (
            tc.tile_pool(name="const", bufs=1) as cpool,
            tc.tile_pool(name="gath", bufs=3) as gpool,
            tc.tile_pool(name="brc", bufs=3) as bpool,
            tc.tile_pool(name="cmp", bufs=3) as mpool,
            tc.tile_pool(name="diff", bufs=2) as dpool,
            tc.tile_pool(name="hid", bufs=4) as hpool,
            tc.tile_pool(name="outp", bufs=1) as opool,
            tc.tile_pool(name="psu", bufs=3, space="PSUM") as upool,
            tc.tile_pool(name="ps1", bufs=3, space="PSUM") as ppool,
            tc.tile_pool(name="ps2", bufs=2, space="PSUM") as p2pool,
        ):
            xpack = cpool.tile([128, N_PADNODES], f16, tag="xpack")
            idxv = cpool.tile([128, E_PAD // 16], i16, tag="idxv")
            iota2 = cpool.tile([128, 2], f16, tag="iota2")
            w1 = cpool.tile([D_FEAT, HID], f16, tag="w1")
            w2two = cpool.tile([128, 2], f16, tag="w2two")
            b1 = cpool.tile([128, 1], f32, tag="b1")
            b2 = cpool.tile([128, 1], f32, tag="b2")
            out_sb = opool.tile([128, COLS], f32, tag="osb")

            nc.sync.dma_start(xpack[:], xpack_d[:])
            nc.sync.dma_start(idxv[:], idxv_d[:])
            nc.sync.dma_start(iota2[:], iota_d[:])
            nc.sync.dma_start(w1[:], w1_d[:])
            nc.sync.dma_start(w2two[:], w2two_d[:])
            nc.sync.dma_start(b1[:], b1_d[:])
            nc.sync.dma_start(b2[:], b2_d[:])

            for w in range(WINDOWS):
                e0 = w * EPW
                gv = gpool.tile([128, EPW], f16, tag="gv")
                nc.gpsimd.dma_gather(
                    gv[:].rearrange("p (a c) -> p a c", a=1),
                    x16_d[:], idxv[:, e0 // 16:(e0 + EPW) // 16],
                    EPW, EPW,
                    elem_size=D_FEAT, transpose=True, single_packet=False,
                )
                # broadcast urel row across partitions (SBUF->SBUF DMA)
                ub = bpool.tile([128, EPW], f16, tag="ub")
                nc.sync.dma_start(
                    ub[:], urel_d[:, e0:e0 + EPW].to_broadcast([128, EPW]),
                )
                # one-hot halves: cmp0 = (urel == p), cmp1 = (urel == p+128)
                cm0 = mpool.tile([128, EPW], f16, tag="cm0")
                cm1 = mpool.tile([128, EPW], f16, tag="cm1")
                nc.vector.tensor_tensor(
                    cm0[:], ub[:], iota2[:, 0:1].to_broadcast([128, EPW]),
                    AluOpType.is_equal)
                nc.vector.tensor_tensor(
                    cm1[:], ub[:], iota2[:, 1:2].to_broadcast([128, EPW]),
                    AluOpType.is_equal)

                blk0 = w * 2  # 128-node block index into xpack
                d = dpool.tile([128, EPW], f16, tag="d")
                for s0 in range(0, EPW, 512):
                    ns = min(512, EPW - s0)
                    pu = upool.tile([128, 512], f32, tag="pu")
                    nc.tensor.matmul(
                        pu[:, 0:ns], xpack[:, blk0 * 128:(blk0 + 1) * 128],
                        cm0[:, s0:s0 + ns], start=True, stop=False,
                    )
                    nc.tensor.matmul(
                        pu[:, 0:ns], xpack[:, (blk0 + 1) * 128:(blk0 + 2) * 128],
                        cm1[:, s0:s0 + ns], start=False, stop=True,
                    )
                    # d = |x[u] - x[v]|
                    nc.vector.tensor_tensor(
                        d[:, s0:s0 + ns], pu[:, 0:ns], gv[:, s0:s0 + ns],
                        AluOpType.subtract)
                    di = d[:, s0:s0 + ns].bitcast(i16)
                    nc.vector.tensor_scalar(
                        di, di, 0x7FFF, None, AluOpType.bitwise_and)

                # MLP over EPW edges: 2 full 1024-groups + tail 128
                ncols_w = 0
                eoff = 0
                col0 = w * BPW  # 17 out cols per window
                while eoff < EPW:
                    na = min(512, EPW - eoff)
                    nb = min(512, EPW - eoff - na)
                    pm = ppool.tile([128, 512], f32, tag="pm")
                    nc.tensor.matmul(
                        pm[0:HID, 0:na], w1[:], d[:, eoff:eoff + na],
                        start=True, stop=True,
                    )
                    if nb:
                        nc.tensor.matmul(
                            pm[HID:128, 0:nb], w1[:],
                            d[:, eoff + na:eoff + na + nb],
                            start=True, stop=True,
                        )
                    h = hpool.tile([128, 512], f16, tag="h")
                    if nb == na:
                        nc.scalar.activation(
                            h[:], pm[:], mybir.ActivationFunctionType.Relu,
                            bias=b1[:], scale=1.0,
                        )
                    else:
                        nc.scalar.activation(
                            h[0:HID, 0:na], pm[0:HID, 0:na],
                            mybir.ActivationFunctionType.Relu,
                            bias=b1[0:HID, :], scale=1.0,
                        )
                    p2 = p2pool.tile([128, 8], f32, tag="p2")
                    np2 = 0
                    for j in range(na // 128):
                        if j < nb // 128:
                            nc.tensor.matmul(
                                p2[:, np2:np2 + 2],
                                h[:, j * 128:(j + 1) * 128], w2two[:],
                                start=True, stop=True,
                            )
                            np2 += 2
                        else:
                            nc.tensor.matmul(
                                p2[:, np2:np2 + 1],
                                h[0:HID, j * 128:(j + 1) * 128],
                                w2two[0:HID, 0:1],
                                start=True, stop=True,
                            )
                            np2 += 1
                    nc.scalar.activation(
                        out_sb[:, col0 + ncols_w:col0 + ncols_w + np2],
                        p2[:, 0:np2],
                        mybir.ActivationFunctionType.Sigmoid,
                        bias=b2[:], scale=1.0,
                    )
                    ncols_w += np2
                    eoff += na + nb

            nc.sync.dma_start(out_d[:], out_sb[:])

    nc.finalize()
    return nc


def _get_nc():
    global _NC_CACHE
    if _NC_CACHE is None:
        _NC_CACHE = _build_nc()
    return _NC_CACHE


def _plan_core(u, v):
    """Sort edges by u, bin into WINDOWS windows padded to EPW each.

    Returns (perm  [E_PAD] int64 edge index into the core's 0..E_CORE-1,
             valid [E_PAD] bool,
             urel  [E_PAD] f16,
             vsel  [E_PAD] int16)."""
    order = np.argsort(u, kind="stable")
    us = u[order]
    win = us // W_WIN
    perm = np.zeros(E_PAD, dtype=np.int64)
    valid = np.zeros(E_PAD, dtype=bool)
    urel = np.zeros(E_PAD, dtype=np.float16)
    vsel = np.zeros(E_PAD, dtype=np.int16)
    starts = np.searchsorted(win, np.arange(WINDOWS + 1))
    for w in range(WINDOWS):
        s, e = starts[w], starts[w + 1]
        cnt = e - s
        assert cnt <= EPW, f"window {w} has {cnt} > {EPW} edges"
        o = w * EPW
        sel = order[s:e]
        perm[o:o + cnt] = sel
        valid[o:o + cnt] = True
        urel[o:o + cnt] = (us[s:e] - w * W_WIN).astype(np.float16)
        vsel[o:o + cnt] = v[sel].astype(np.int16)
        # padding: urel stays 0 -> gathers x[w*W_WIN], v stays 0
    return perm, valid, urel, vsel


def prep_in_maps(x, indices, W1, b1, W2, b2):
    x16 = np.asarray(x, dtype=np.float32).astype(np.float16)
    idx = np.asarray(indices)
    xpack = np.zeros((128, N_PADNODES), dtype=np.float16)
    for r in range(N_PADNODES // 128):
        n0, n1 = r * 128, min(r * 128 + 128, N_NODES)
        if n1 > n0:
            xpack[: n1 - n0, r * 128:r * 128 + 128] = x16[n0:n1]
    iota2 = np.zeros((128, 2), dtype=np.float16)
    iota2[:, 0] = np.arange(128)
    iota2[:, 1] = np.arange(128) + 128

    w1 = np.asarray(W1, dtype=np.float32).astype(np.float16)
    w2c = np.asarray(W2, dtype=np.float32).astype(np.float16).reshape(HID)
    w2two = np.zeros((128, 2), dtype=np.float16)
    w2two[0:HID, 0] = w2c
    w2two[HID:128, 1] = w2c
    b1c = np.asarray(b1, dtype=np.float32).reshape(HID, 1)
    b1s = np.concatenate([b1c, b1c], axis=0)
    b2s = np.full((128, 1), np.asarray(b2, dtype=np.float32).reshape(-1)[0],
                  dtype=np.float32)

    in_maps = []
    plans = []
    for c in range(N_CORES):
        sl = slice(c * E_CORE, (c + 1) * E_CORE)
        u = np.asarray(idx[0, sl], dtype=np.int64)
        v = np.asarray(idx[1, sl], dtype=np.int64)
        perm, valid, urel, vsel = _plan_core(u, v)
        plans.append((perm, valid))
        idxv = np.tile(vsel.reshape(-1, 16).T, (8, 1)).astype(np.int16)
        in_maps.append({
            "x16": x16, "xpack": xpack,
            "idxv": idxv, "urel": urel.reshape(1, E_PAD),
            "iota2": iota2, "w1": w1, "w2two": w2two,
            "b1": b1s, "b2": b2s,
        })
    return in_maps, plans


def _edge_of_col():
    """Map out_sb column -> padded-edge base (partition p adds offsets)."""
    # column layout per window: 17 cols; groups of (4A,4B interleaved) per
    # 1024-edge supergroup, then the 128-edge tail as one col.
    base = np.zeros(COLS, dtype=np.int64)
    for w in range(WINDOWS):
        col = w * BPW
        eoff = 0
        while eoff < EPW:
            na = min(512, EPW - eoff)
            nb = min(512, EPW - eoff - na)
            for j in range(na // 128):
                if j < nb // 128:
                    base[col] = w * EPW + eoff + j * 128
                    base[col + 1] = w * EPW + eoff + na + j * 128
                    col += 2
                else:
                    base[col] = w * EPW + eoff + j * 128
                    col += 1
            eoff += na + nb
    return base


def run_hw(x, indices, W1, b1, W2, b2, trace=False, **kw):
    nc = _get_nc()
    in_maps, plans = prep_in_maps(x, indices, W1, b1, W2, b2)
    res = run_bass_kernel_spmd(
        nc, in_maps, core_ids=list(range(N_CORES)), trace=trace, **kw
    )
    base = _edge_of_col()
    pedge = base[None, :] + np.arange(128)[:, None]  # [128, COLS] padded idx
    out = np.empty(N_EDGES, dtype=np.float32)
    for c in range(N_CORES):
        o = np.asarray(res.results[c]["out"])  # [128, COLS]
        perm, valid = plans[c]
        flat = np.empty(E_PAD, dtype=np.float32)
        flat[pedge.ravel()] = o.ravel()
        out[c * E_CORE + perm[valid]] = flat[valid]
    return out, res


def kernel(x, indices, W1, b1, W2, b2):
    out, _ = run_hw(x, indices, W1, b1, W2, b2, trace=False)
    return out.astype(np.float32)


# revision 3
# speedup vs baseline: 1.1728x; 1.0617x over previous
"""Edge-MLP GNN message passing kernel for Trainium2 (8 NeuronCores), v5.

out[e] = sigmoid(relu(|x[u_e] - x[v_e]| @ W1 + b1) @ W2 + b2)

v5 = v4 with the padding and overheads squeezed out:
  - Per-window edge counts are EXACT compile-time constants: the kernel is
    built after seeing the indices, so window w is padded only to
    round128(max over the 8 cores of count_w) instead of a uniform
    worst-case pad (92160 -> ~85k indices; the Q7 descriptor generator at
    ~8.5ns/idx is the bottleneck, so fewer indices = directly faster).
  - v-side dma_gather calls are batched over consecutive windows (up to
    7680 indices/call, ring-safe) to amortize per-call SWDGE overhead.
  - The one-hot is_equal compares use materialized iota tiles (both
    operands packed f16) so DVE runs them in 2x mode.
  - Everything else as v4: u-side gather via PE one-hot matmuls over
    256-node windows, d = |psum_u - g_v| (DVE subtract + i16 sign mask),
    mm1 packed halves, ACT relu, mm2 w2two-paired, ACT sigmoid.
"""

import os
import sys

for _p in ("/opt/trn_rl_repo", "/root/.axon_site/_ro/trn_rl_repo"):
    if os.path.isdir(_p) and _p not in sys.path:
        sys.path.insert(0, _p)

import numpy as np

import concourse.bacc as bacc
import concourse.mybir as mybir
from concourse.mybir import AluOpType
from concourse.tile import TileContext
from concourse.bass_utils import run_bass_kernel_spmd

N_NODES = 10000
N_EDGES = 640000
D_FEAT = 128
HID = 64
N_CORES = 8
E_CORE = N_EDGES // N_CORES      # 80000

W_WIN = 256
WINDOWS = 40                     # 40 * 256 = 10240 node-id range
N_PADNODES = WINDOWS * W_WIN
MAX_EPW = 2304                   # tile sizing bound for one window
BATCH_MAX = 2304                 # max gather indices per SWDGE call (per-window)

f16 = mybir.dt.float16
f32 = mybir.dt.float32
i16 = mybir.dt.int16

_NC_CACHE = {}


def _plan_batches(epws):
    """Group consecutive windows into gather batches of <= BATCH_MAX idxs."""
    batches = []
    cur = []
    tot = 0
    for w, n in enumerate(epws):
        if n == 0:
            continue
        if tot + n > BATCH_MAX and cur:
            batches.append(cur)
            cur, tot = [], 0
        cur.append(w)
        tot += n
    if cur:
        batches.append(cur)
    return batches


def _col_layout(epws):
    """Per-column padded-edge base, replicating the kernel's emission order."""
    base = []
    eoff_w = 0
    for n in epws:
        eoff = 0
        while eoff < n:
            na = min(512, n - eoff)
            nb = min(512, n - eoff - na)
            for j in range(na // 128):
                if j < nb // 128:
                    base.append(eoff_w + eoff + j * 128)
                    base.append(eoff_w + eoff + na + j * 128)
                else:
                    base.append(eoff_w + eoff + j * 128)
            eoff += na + nb
        eoff_w += n
    return np.asarray(base, dtype=np.int64)


def _build_nc(epws):
    e_pad = sum(epws)
    cols = e_pad // 128
    woff = np.concatenate([[0], np.cumsum(epws)]).astype(np.int64)

    nc = bacc.Bacc("TRN2", target_bir_lowering=False)

    x16_d = nc.dram_tensor("x16", [N_NODES, D_FEAT], f16, kind="ExternalInput")
    xpack_d = nc.dram_tensor("xpack", [128, N_PADNODES], f16,
                             kind="ExternalInput")
    idxv_d = nc.dram_tensor("idxv", [128, e_pad // 16], i16,
                            kind="ExternalInput")
    urel_d = nc.dram_tensor("urel", [1, e_pad], f16, kind="ExternalInput")
    iota_d = nc.dram_tensor("iotat", [128, 2 * MAX_EPW], f16,
                            kind="ExternalInput")
    w1_d = nc.dram_tensor("w1", [D_FEAT, HID], f16, kind="ExternalInput")
    w2two_d = nc.dram_tensor("w2two", [128, 2], f16, kind="ExternalInput")
    b1_d = nc.dram_tensor("b1", [128, 1], f32, kind="ExternalInput")
    b2_d = nc.dram_tensor("b2", [128, 1], f32, kind="ExternalInput")
    out_d = nc.dram_tensor("out", [128, cols], f32, kind="ExternalOutput")

    batches = _plan_batches(epws)

    with TileContext(nc) as tc:
        with (
            tc.tile_pool(name="const", bufs=1) as cpool,
            tc.tile_pool(name="gath", bufs=2) as gpool,
            tc.tile_pool(name="brc", bufs=3) as bpool,
            tc.tile_pool(name="cmp", bufs=3) as mpool,
            tc.tile_pool(name="diff", bufs=2) as dpool,
            tc.tile_pool(name="d2p", bufs=3) as d2pool,
            tc.tile_pool(name="hid", bufs=4) as hpool,
            tc.tile_pool(name="outp", bufs=1) as opool,
            tc.tile_pool(name="psu", bufs=3, space="PSUM") as upool,
            tc.tile_pool(name="ps1", bufs=3, space="PSUM") as ppool,
            tc.tile_pool(name="ps2", bufs=2, space="PSUM") as p2pool,
        ):
            xpack = cpool.tile([128, N_PADNODES], f16, tag="xpack")
            idxv = cpool.tile([128, e_pad // 16], i16, tag="idxv")
            iotat = cpool.tile([128, 2 * MAX_EPW], f16, tag="iotat")
            w1 = cpool.tile([D_FEAT, HID], f16, tag="w1")
            w2two = cpool.tile([128, 2], f16, tag="w2two")
            b1 = cpool.tile([128, 1], f32, tag="b1")
            b2 = cpool.tile([128, 1], f32, tag="b2")
            out_sb = opool.tile([128, cols], f32, tag="osb")

            nc.sync.dma_start(xpack[:], xpack_d[:])
            nc.sync.dma_start(idxv[:], idxv_d[:])
            nc.sync.dma_start(iotat[:], iota_d[:])
            nc.sync.dma_start(w1[:], w1_d[:])
            nc.sync.dma_start(w2two[:], w2two_d[:])
            nc.sync.dma_start(b1[:], b1_d[:])
            nc.sync.dma_start(b2[:], b2_d[:])

            col = 0
            for batch in batches:
                b0 = int(woff[batch[0]])
                blen = int(woff[batch[-1] + 1] - b0)
                gv = gpool.tile([128, BATCH_MAX], f16, tag="gv")
                nc.gpsimd.dma_gather(
                    gv[:, 0:blen].rearrange("p (a c) -> p a c", a=1),
                    x16_d[:], idxv[:, b0 // 16:(b0 + blen) // 16],
                    blen, blen,
                    elem_size=D_FEAT, transpose=True, single_packet=False,
                )
                for w in batch:
                    n = epws[w]
                    e0 = int(woff[w])
                    g0 = e0 - b0  # offset within gv
                    ub = bpool.tile([128, MAX_EPW], f16, tag="ub")
                    nc.sync.dma_start(
                        ub[:, 0:n],
                        urel_d[:, e0:e0 + n].to_broadcast([128, n]),
                    )
                    cm0 = mpool.tile([128, MAX_EPW], f16, tag="cm0")
                    cm1 = mpool.tile([128, MAX_EPW], f16, tag="cm1")
                    nc.vector.tensor_tensor(
                        cm0[:, 0:n], ub[:, 0:n], iotat[:, 0:n],
                        AluOpType.is_equal)
                    nc.vector.tensor_tensor(
                        cm1[:, 0:n], ub[:, 0:n],
                        iotat[:, MAX_EPW:MAX_EPW + n],
                        AluOpType.is_equal)

                    blk0 = w * 2
                    d = dpool.tile([128, MAX_EPW], f16, tag="d")
                    for s0 in range(0, n, 512):
                        ns = min(512, n - s0)
                        pu = upool.tile([128, 512], f32, tag="pu")
                        nc.tensor.matmul(
                            pu[:, 0:ns],
                            xpack[:, blk0 * 128:(blk0 + 1) * 128],
                            cm0[:, s0:s0 + ns], start=True, stop=False,
                        )
                        nc.tensor.matmul(
                            pu[:, 0:ns],
                            xpack[:, (blk0 + 1) * 128:(blk0 + 2) * 128],
                            cm1[:, s0:s0 + ns], start=False, stop=True,
                        )
                        d2 = d2pool.tile([128, 512], f16, tag="d2")
                        nc.vector.tensor_tensor(
                            d[:, s0:s0 + ns], pu[:, 0:ns],
                            gv[:, g0 + s0:g0 + s0 + ns],
                            AluOpType.subtract)
                        nc.vector.tensor_tensor(
                            d2[:, 0:ns], gv[:, g0 + s0:g0 + s0 + ns],
                            pu[:, 0:ns], AluOpType.subtract)
                        nc.vector.tensor_tensor(
                            d[:, s0:s0 + ns], d[:, s0:s0 + ns], d2[:, 0:ns],
                            AluOpType.max)

                    eoff = 0
                    while eoff < n:
                        na = min(512, n - eoff)
                        nb = min(512, n - eoff - na)
                        pm = ppool.tile([128, 512], f32, tag="pm")
                        nc.tensor.matmul(
                            pm[0:HID, 0:na], w1[:], d[:, eoff:eoff + na],
                            start=True, stop=True,
                        )
                        if nb:
                            nc.tensor.matmul(
                                pm[HID:128, 0:nb], w1[:],
                                d[:, eoff + na:eoff + na + nb],
                                start=True, stop=True,
                            )
                        h = hpool.tile([128, 512], f16, tag="h")
                        if nb == na:
                            nc.scalar.activation(
                                h[:], pm[:],
                                mybir.ActivationFunctionType.Relu,
                                bias=b1[:], scale=1.0,
                            )
                        else:
                            nc.scalar.activation(
                                h[0:HID, 0:na], pm[0:HID, 0:na],
                                mybir.ActivationFunctionType.Relu,
                                bias=b1[0:HID, :], scale=1.0,
                            )
                        p2 = p2pool.tile([128, 8], f32, tag="p2")
                        np2 = 0
                        for j in range(na // 128):
                            if j < nb // 128:
                                nc.tensor.matmul(
                                    p2[:, np2:np2 + 2],
                                    h[:, j * 128:(j + 1) * 128], w2two[:],
                                    start=True, stop=True,
                                )
                                np2 += 2
                            else:
                                nc.tensor.matmul(
                                    p2[:, np2:np2 + 1],
                                    h[0:HID, j * 128:(j + 1) * 128],
                                    w2two[0:HID, 0:1],
                                    start=True, stop=True,
                                )
                                np2 += 1
                        nc.scalar.activation(
                            out_sb[:, col:col + np2], p2[:, 0:np2],
                            mybir.ActivationFunctionType.Sigmoid,
                            bias=b2[:], scale=1.0,
                        )
                        col += np2
                        eoff += na + nb

            assert col == cols
            nc.sync.dma_start(out_d[:], out_sb[:])

    nc.finalize()
    return nc


def _get_nc(epws):
    key = tuple(epws)
    if key not in _NC_CACHE:
        _NC_CACHE[key] = _build_nc(key)
    return _NC_CACHE[key]


def _window_sizes(idx):
    """round128(max over cores of per-window count)."""
    counts = np.zeros((N_CORES, WINDOWS), dtype=np.int64)
    for c in range(N_CORES):
        u = np.asarray(idx[0, c * E_CORE:(c + 1) * E_CORE], dtype=np.int64)
        counts[c] = np.bincount(u // W_WIN, minlength=WINDOWS)
    mx = counts.max(axis=0)
    epws = ((mx + 127) // 128) * 128
    assert epws.max() <= MAX_EPW
    return tuple(int(x) for x in epws)


def _plan_core(u, v, epws):
    e_pad = sum(epws)
    woff = np.concatenate([[0], np.cumsum(epws)]).astype(np.int64)
    order = np.argsort(u, kind="stable")
    us = u[order]
    win = us // W_WIN
    perm = np.zeros(e_pad, dtype=np.int64)
    valid = np.zeros(e_pad, dtype=bool)
    urel = np.zeros(e_pad, dtype=np.float16)
    vsel = np.zeros(e_pad, dtype=np.int16)
    starts = np.searchsorted(win, np.arange(WINDOWS + 1))
    for w in range(WINDOWS):
        s, e = starts[w], starts[w + 1]
        cnt = e - s
        assert cnt <= epws[w]
        o = int(woff[w])
        sel = order[s:e]
        perm[o:o + cnt] = sel
        valid[o:o + cnt] = True
        urel[o:o + cnt] = (us[s:e] - w * W_WIN).astype(np.float16)
        vsel[o:o + cnt] = v[sel].astype(np.int16)
    return perm, valid, urel, vsel


def prep_in_maps(x, indices, W1, b1, W2, b2, epws):
    x16 = np.asarray(x, dtype=np.float32).astype(np.float16)
    idx = np.asarray(indices)
    e_pad = sum(epws)
    xpack = np.zeros((128, N_PADNODES), dtype=np.float16)
    for r in range(N_PADNODES // 128):
        n0, n1 = r * 128, min(r * 128 + 128, N_NODES)
        if n1 > n0:
            xpack[: n1 - n0, r * 128:r * 128 + 128] = x16[n0:n1]
    iotat = np.zeros((128, 2 * MAX_EPW), dtype=np.float16)
    iotat[:, 0:MAX_EPW] = np.arange(128, dtype=np.float16)[:, None]
    iotat[:, MAX_EPW:] = (np.arange(128) + 128).astype(np.float16)[:, None]

    w1 = np.asarray(W1, dtype=np.float32).astype(np.float16)
    w2c = np.asarray(W2, dtype=np.float32).astype(np.float16).reshape(HID)
    w2two = np.zeros((128, 2), dtype=np.float16)
    w2two[0:HID, 0] = w2c
    w2two[HID:128, 1] = w2c
    b1c = np.asarray(b1, dtype=np.float32).reshape(HID, 1)
    b1s = np.concatenate([b1c, b1c], axis=0)
    b2s = np.full((128, 1), np.asarray(b2, dtype=np.float32).reshape(-1)[0],
                  dtype=np.float32)

    in_maps = []
    plans = []
    for c in range(N_CORES):
        sl = slice(c * E_CORE, (c + 1) * E_CORE)
        u = np.asarray(idx[0, sl], dtype=np.int64)
        v = np.asarray(idx[1, sl], dtype=np.int64)
        perm, valid, urel, vsel = _plan_core(u, v, epws)
        plans.append((perm, valid))
        idxv = np.tile(vsel.reshape(-1, 16).T, (8, 1)).astype(np.int16)
        in_maps.append({
            "x16": x16, "xpack": xpack,
            "idxv": idxv, "urel": urel.reshape(1, e_pad),
            "iotat": iotat, "w1": w1, "w2two": w2two,
            "b1": b1s, "b2": b2s,
        })
    return in_maps, plans


def run_hw(x, indices, W1, b1, W2, b2, trace=False, **kw):
    epws = _window_sizes(np.asarray(indices))
    nc = _get_nc(epws)
    in_maps, plans = prep_in_maps(x, indices, W1, b1, W2, b2, epws)
    res = run_bass_kernel_spmd(
        nc, in_maps, core_ids=list(range(N_CORES)), trace=trace, **kw
    )
    base = _col_layout(epws)
    pedge = base[None, :] + np.arange(128)[:, None]
    e_pad = sum(epws)
    out = np.empty(N_EDGES, dtype=np.float32)
    for c in range(N_CORES):
        o = np.asarray(res.results[c]["out"])
        perm, valid = plans[c]
        flat = np.empty(e_pad, dtype=np.float32)
        flat[pedge.ravel()] = o.ravel()
        out[c * E_CORE + perm[valid]] = flat[valid]
    return out, res


def kernel(x, indices, W1, b1, W2, b2):
    out, _ = run_hw(x, indices, W1, b1, W2, b2, trace=False)
    return out.astype(np.float32)


# revision 4
# speedup vs baseline: 1.1749x; 1.0018x over previous
"""Edge-MLP GNN message passing kernel for Trainium2 (8 NeuronCores), v5.

out[e] = sigmoid(relu(|x[u_e] - x[v_e]| @ W1 + b1) @ W2 + b2)

v6 = v5 + zero-padding edge assignment:
  - Each edge may be processed in the window of EITHER endpoint (|x_u - x_v|
    is symmetric): a host-side greedy rebalance assigns edges to windows so
    every window's global count is an exact multiple of 1024 (= 128 cols x 8
    cores), then deals each window's edges evenly across the 8 cores. The
    v-side gather stream is exactly 80000 indices per core - no padding.

v5 was v4 with the padding and overheads squeezed out:
  - Per-window edge counts are EXACT compile-time constants: the kernel is
    built after seeing the indices, so window w is padded only to
    round128(max over the 8 cores of count_w) instead of a uniform
    worst-case pad (92160 -> ~85k indices; the Q7 descriptor generator at
    ~8.5ns/idx is the bottleneck, so fewer indices = directly faster).
  - v-side dma_gather calls are batched over consecutive windows (up to
    7680 indices/call, ring-safe) to amortize per-call SWDGE overhead.
  - The one-hot is_equal compares use materialized iota tiles (both
    operands packed f16) so DVE runs them in 2x mode.
  - Everything else as v4: u-side gather via PE one-hot matmuls over
    256-node windows, d = |psum_u - g_v| (DVE subtract + i16 sign mask),
    mm1 packed halves, ACT relu, mm2 w2two-paired, ACT sigmoid.
"""

import os
import sys

for _p in ("/opt/trn_rl_repo", "/root/.axon_site/_ro/trn_rl_repo"):
    if os.path.isdir(_p) and _p not in sys.path:
        sys.path.insert(0, _p)

import numpy as np

import concourse.bacc as bacc
import concourse.mybir as mybir
from concourse.mybir import AluOpType
from concourse.tile import TileContext
from concourse.bass_utils import run_bass_kernel_spmd

N_NODES = 10000
N_EDGES = 640000
D_FEAT = 128
HID = 64
N_CORES = 8
E_CORE = N_EDGES // N_CORES      # 80000

W_WIN = 256
WINDOWS = 40                     # 40 * 256 = 10240 node-id range
N_PADNODES = WINDOWS * W_WIN
MAX_EPW = 2304                   # tile sizing bound for one window
BATCH_MAX = 2304                 # max gather indices per SWDGE call (per-window)

f16 = mybir.dt.float16
f32 = mybir.dt.float32
i16 = mybir.dt.int16

_NC_CACHE = {}


def _plan_batches(epws):
    """Group consecutive windows into gather batches of <= BATCH_MAX idxs."""
    batches = []
    cur = []
    tot = 0
    for w, n in enumerate(epws):
        if n == 0:
            continue
        if tot + n > BATCH_MAX and cur:
            batches.append(cur)
            cur, tot = [], 0
        cur.append(w)
        tot += n
    if cur:
        batches.append(cur)
    return batches


def _col_layout(epws):
    """Per-column padded-edge base, replicating the kernel's emission order."""
    base = []
    eoff_w = 0
    for n in epws:
        eoff = 0
        while eoff < n:
            na = min(512, n - eoff)
            nb = min(512, n - eoff - na)
            for j in range(na // 128):
                if j < nb // 128:
                    base.append(eoff_w + eoff + j * 128)
                    base.append(eoff_w + eoff + na + j * 128)
                else:
                    base.append(eoff_w + eoff + j * 128)
            eoff += na + nb
        eoff_w += n
    return np.asarray(base, dtype=np.int64)


def _build_nc(epws):
    e_pad = sum(epws)
    cols = e_pad // 128
    woff = np.concatenate([[0], np.cumsum(epws)]).astype(np.int64)

    nc = bacc.Bacc("TRN2", target_bir_lowering=False)

    x16_d = nc.dram_tensor("x16", [N_NODES, D_FEAT], f16, kind="ExternalInput")
    xpack_d = nc.dram_tensor("xpack", [128, N_PADNODES], f16,
                             kind="ExternalInput")
    idxv_d = nc.dram_tensor("idxv", [128, e_pad // 16], i16,
                            kind="ExternalInput")
    urel_d = nc.dram_tensor("urel", [1, e_pad], f16, kind="ExternalInput")
    iota_d = nc.dram_tensor("iotat", [128, 2 * MAX_EPW], f16,
                            kind="ExternalInput")
    w1_d = nc.dram_tensor("w1", [D_FEAT, HID], f16, kind="ExternalInput")
    w2two_d = nc.dram_tensor("w2two", [128, 2], f16, kind="ExternalInput")
    b1_d = nc.dram_tensor("b1", [128, 1], f32, kind="ExternalInput")
    b2_d = nc.dram_tensor("b2", [128, 1], f32, kind="ExternalInput")
    out_d = nc.dram_tensor("out", [128, cols], f32, kind="ExternalOutput")

    batches = _plan_batches(epws)

    with TileContext(nc) as tc:
        with (
            tc.tile_pool(name="const", bufs=1) as cpool,
            tc.tile_pool(name="gath", bufs=2) as gpool,
            tc.tile_pool(name="brc", bufs=3) as bpool,
            tc.tile_pool(name="cmp", bufs=3) as mpool,
            tc.tile_pool(name="diff", bufs=2) as dpool,
            tc.tile_pool(name="d2p", bufs=3) as d2pool,
            tc.tile_pool(name="hid", bufs=4) as hpool,
            tc.tile_pool(name="outp", bufs=1) as opool,
            tc.tile_pool(name="psu", bufs=3, space="PSUM") as upool,
            tc.tile_pool(name="ps1", bufs=3, space="PSUM") as ppool,
            tc.tile_pool(name="ps2", bufs=2, space="PSUM") as p2pool,
        ):
            xpack = cpool.tile([128, N_PADNODES], f16, tag="xpack")
            idxv = cpool.tile([128, e_pad // 16], i16, tag="idxv")
            iotat = cpool.tile([128, 2 * MAX_EPW], f16, tag="iotat")
            w1 = cpool.tile([D_FEAT, HID], f16, tag="w1")
            w2two = cpool.tile([128, 2], f16, tag="w2two")
            b1 = cpool.tile([128, 1], f32, tag="b1")
            b2 = cpool.tile([128, 1], f32, tag="b2")
            out_sb = opool.tile([128, cols], f32, tag="osb")

            nc.sync.dma_start(xpack[:], xpack_d[:])
            nc.sync.dma_start(idxv[:], idxv_d[:])
            nc.sync.dma_start(iotat[:], iota_d[:])
            nc.sync.dma_start(w1[:], w1_d[:])
            nc.sync.dma_start(w2two[:], w2two_d[:])
            nc.sync.dma_start(b1[:], b1_d[:])
            nc.sync.dma_start(b2[:], b2_d[:])

            col = 0
            for batch in batches:
                b0 = int(woff[batch[0]])
                blen = int(woff[batch[-1] + 1] - b0)
                gv = gpool.tile([128, BATCH_MAX], f16, tag="gv")
                nc.gpsimd.dma_gather(
                    gv[:, 0:blen].rearrange("p (a c) -> p a c", a=1),
                    x16_d[:], idxv[:, b0 // 16:(b0 + blen) // 16],
                    blen, blen,
                    elem_size=D_FEAT, transpose=True, single_packet=False,
                )
                for w in batch:
                    n = epws[w]
                    e0 = int(woff[w])
                    g0 = e0 - b0  # offset within gv
                    ub = bpool.tile([128, MAX_EPW], f16, tag="ub")
                    nc.sync.dma_start(
                        ub[:, 0:n],
                        urel_d[:, e0:e0 + n].to_broadcast([128, n]),
                    )
                    cm0 = mpool.tile([128, MAX_EPW], f16, tag="cm0")
                    cm1 = mpool.tile([128, MAX_EPW], f16, tag="cm1")
                    nc.vector.tensor_tensor(
                        cm0[:, 0:n], ub[:, 0:n], iotat[:, 0:n],
                        AluOpType.is_equal)
                    nc.vector.tensor_tensor(
                        cm1[:, 0:n], ub[:, 0:n],
                        iotat[:, MAX_EPW:MAX_EPW + n],
                        AluOpType.is_equal)

                    blk0 = w * 2
                    d = dpool.tile([128, MAX_EPW], f16, tag="d")
                    for s0 in range(0, n, 512):
                        ns = min(512, n - s0)
                        pu = upool.tile([128, 512], f32, tag="pu")
                        nc.tensor.matmul(
                            pu[:, 0:ns],
                            xpack[:, blk0 * 128:(blk0 + 1) * 128],
                            cm0[:, s0:s0 + ns], start=True, stop=False,
                        )
                        nc.tensor.matmul(
                            pu[:, 0:ns],
                            xpack[:, (blk0 + 1) * 128:(blk0 + 2) * 128],
                            cm1[:, s0:s0 + ns], start=False, stop=True,
                        )
                        d2 = d2pool.tile([128, 512], f16, tag="d2")
                        nc.vector.tensor_tensor(
                            d[:, s0:s0 + ns], pu[:, 0:ns],
                            gv[:, g0 + s0:g0 + s0 + ns],
                            AluOpType.subtract)
                        nc.vector.tensor_tensor(
                            d2[:, 0:ns], gv[:, g0 + s0:g0 + s0 + ns],
                            pu[:, 0:ns], AluOpType.subtract)
                        nc.vector.tensor_tensor(
                            d[:, s0:s0 + ns], d[:, s0:s0 + ns], d2[:, 0:ns],
                            AluOpType.max)

                    eoff = 0
                    while eoff < n:
                        na = min(512, n - eoff)
                        nb = min(512, n - eoff - na)
                        pm = ppool.tile([128, 512], f32, tag="pm")
                        nc.tensor.matmul(
                            pm[0:HID, 0:na], w1[:], d[:, eoff:eoff + na],
                            start=True, stop=True,
                        )
                        if nb:
                            nc.tensor.matmul(
                                pm[HID:128, 0:nb], w1[:],
                                d[:, eoff + na:eoff + na + nb],
                                start=True, stop=True,
                            )
                        h = hpool.tile([128, 512], f16, tag="h")
                        if nb == na:
                            nc.scalar.activation(
                                h[:], pm[:],
                                mybir.ActivationFunctionType.Relu,
                                bias=b1[:], scale=1.0,
                            )
                        else:
                            nc.scalar.activation(
                                h[0:HID, 0:na], pm[0:HID, 0:na],
                                mybir.ActivationFunctionType.Relu,
                                bias=b1[0:HID, :], scale=1.0,
                            )
                        p2 = p2pool.tile([128, 8], f32, tag="p2")
                        np2 = 0
                        for j in range(na // 128):
                            if j < nb // 128:
                                nc.tensor.matmul(
                                    p2[:, np2:np2 + 2],
                                    h[:, j * 128:(j + 1) * 128], w2two[:],
                                    start=True, stop=True,
                                )
                                np2 += 2
                            else:
                                nc.tensor.matmul(
                                    p2[:, np2:np2 + 1],
                                    h[0:HID, j * 128:(j + 1) * 128],
                                    w2two[0:HID, 0:1],
                                    start=True, stop=True,
                                )
                                np2 += 1
                        nc.scalar.activation(
                            out_sb[:, col:col + np2], p2[:, 0:np2],
                            mybir.ActivationFunctionType.Sigmoid,
                            bias=b2[:], scale=1.0,
                        )
                        col += np2
                        eoff += na + nb

            assert col == cols
            nc.sync.dma_start(out_d[:], out_sb[:])

    nc.finalize()
    return nc


def _get_nc(epws):
    key = tuple(epws)
    if key not in _NC_CACHE:
        _NC_CACHE[key] = _build_nc(key)
    return _NC_CACHE[key]


def _plan_global(idx):
    """Assign each edge to the window of one endpoint so that every window's
    global count is a multiple of 1024, then deal evenly across cores.

    Returns (epws, per_core_plans) where per_core_plans[c] =
    (edges [e_core_pad] global edge ids, side [e_core_pad] 0=u-window,
     valid mask)."""
    u = np.asarray(idx[0], dtype=np.int64)
    v = np.asarray(idx[1], dtype=np.int64)
    wu = u // W_WIN
    wv = v // W_WIN
    side = np.zeros(N_EDGES, dtype=np.int8)
    cnt = np.bincount(wu, minlength=WINDOWS).astype(np.int64)

    # targets: multiples of 1024 summing to N_EDGES, tracking natural counts
    t = (cnt // 1024) * 1024
    rem = (N_EDGES - t.sum()) // 1024
    order = np.argsort(-(cnt - t))  # windows losing most get the spare units
    t[order[:rem]] += 1024
    assert t.sum() == N_EDGES and (t % 1024 == 0).all()

    # greedy repair: move flexible edges from over-full to under-full windows
    bywin = [list(np.nonzero(wu == w)[0]) for w in range(WINDOWS)]
    for _ in range(12):
        need = cnt - t
        if not need.any():
            break
        for w in range(WINDOWS):
            while need[w] > 0 and bywin[w]:
                moved = False
                for k in range(len(bywin[w]) - 1, -1, -1):
                    e = bywin[w][k]
                    tgt = wv[e]
                    if tgt != w and need[tgt] < 0:
                        bywin[w].pop(k)
                        side[e] = 1
                        need[w] -= 1
                        need[tgt] += 1
                        cnt[w] -= 1
                        cnt[tgt] += 1
                        moved = True
                        if need[w] <= 0:
                            break
                if not moved:
                    break
    assert (cnt == t).all(), f"rebalance failed: {cnt - t}"

    # collect per-window edge lists (u-side edges + moved v-side edges)
    win_of = np.where(side == 0, wu, wv)
    epws = tuple(int(x) // N_CORES for x in t)  # per-core window sizes
    e_core = sum(epws)
    assert e_core == N_EDGES // N_CORES

    plans = []
    ordglob = np.argsort(win_of, kind="stable")
    starts = np.searchsorted(win_of[ordglob], np.arange(WINDOWS + 1))
    percore_edges = np.empty((N_CORES, e_core), dtype=np.int64)
    woff = np.concatenate([[0], np.cumsum(epws)]).astype(np.int64)
    for w in range(WINDOWS):
        es = ordglob[starts[w]:starts[w + 1]]
        n = epws[w]
        for c in range(N_CORES):
            percore_edges[c, woff[w]:woff[w] + n] = es[c * n:(c + 1) * n]
    for c in range(N_CORES):
        plans.append((percore_edges[c], side[percore_edges[c]]))
    return epws, plans


def prep_in_maps(x, indices, W1, b1, W2, b2, epws, plans):
    x16 = np.asarray(x, dtype=np.float32).astype(np.float16)
    idx = np.asarray(indices)
    u = np.asarray(idx[0], dtype=np.int64)
    v = np.asarray(idx[1], dtype=np.int64)
    e_pad = sum(epws)
    woff = np.concatenate([[0], np.cumsum(epws)]).astype(np.int64)
    xpack = np.zeros((128, N_PADNODES), dtype=np.float16)
    for r in range(N_PADNODES // 128):
        n0, n1 = r * 128, min(r * 128 + 128, N_NODES)
        if n1 > n0:
            xpack[: n1 - n0, r * 128:r * 128 + 128] = x16[n0:n1]
    iotat = np.zeros((128, 2 * MAX_EPW), dtype=np.float16)
    iotat[:, 0:MAX_EPW] = np.arange(128, dtype=np.float16)[:, None]
    iotat[:, MAX_EPW:] = (np.arange(128) + 128).astype(np.float16)[:, None]

    w1 = np.asarray(W1, dtype=np.float32).astype(np.float16)
    w2c = np.asarray(W2, dtype=np.float32).astype(np.float16).reshape(HID)
    w2two = np.zeros((128, 2), dtype=np.float16)
    w2two[0:HID, 0] = w2c
    w2two[HID:128, 1] = w2c
    b1c = np.asarray(b1, dtype=np.float32).reshape(HID, 1)
    b1s = np.concatenate([b1c, b1c], axis=0)
    b2s = np.full((128, 1), np.asarray(b2, dtype=np.float32).reshape(-1)[0],
                  dtype=np.float32)

    in_maps = []
    for c in range(N_CORES):
        edges, sd = plans[c]
        # PE-side node (one-hot) and Q7-side node (dma_gather)
        pe_node = np.where(sd == 0, u[edges], v[edges])
        q7_node = np.where(sd == 0, v[edges], u[edges])
        urel = np.zeros(e_pad, dtype=np.float16)
        for w in range(WINDOWS):
            s, e = int(woff[w]), int(woff[w + 1])
            urel[s:e] = (pe_node[s:e] - w * W_WIN).astype(np.float16)
        vsel = q7_node.astype(np.int16)
        idxv = np.tile(vsel.reshape(-1, 16).T, (8, 1)).astype(np.int16)
        in_maps.append({
            "x16": x16, "xpack": xpack,
            "idxv": idxv, "urel": urel.reshape(1, e_pad),
            "iotat": iotat, "w1": w1, "w2two": w2two,
            "b1": b1s, "b2": b2s,
        })
    return in_maps


def run_hw(x, indices, W1, b1, W2, b2, trace=False, **kw):
    epws, plans = _plan_global(np.asarray(indices))
    nc = _get_nc(epws)
    in_maps = prep_in_maps(x, indices, W1, b1, W2, b2, epws, plans)
    res = run_bass_kernel_spmd(
        nc, in_maps, core_ids=list(range(N_CORES)), trace=trace, **kw
    )
    base = _col_layout(epws)
    pedge = base[None, :] + np.arange(128)[:, None]
    e_pad = sum(epws)
    out = np.empty(N_EDGES, dtype=np.float32)
    for c in range(N_CORES):
        o = np.asarray(res.results[c]["out"])
        edges, _sd = plans[c]
        flat = np.empty(e_pad, dtype=np.float32)
        flat[pedge.ravel()] = o.ravel()
        out[edges] = flat
    return out, res


def kernel(x, indices, W1, b1, W2, b2):
    out, _ = run_hw(x, indices, W1, b1, W2, b2, trace=False)
    return out.astype(np.float32)


# revision 5
# speedup vs baseline: 1.2883x; 1.0965x over previous
"""Edge-MLP GNN message passing kernel for Trainium2 (8 NeuronCores), v5.

out[e] = sigmoid(relu(|x[u_e] - x[v_e]| @ W1 + b1) @ W2 + b2)

v6 = v5 + zero-padding edge assignment:
  - Each edge may be processed in the window of EITHER endpoint (|x_u - x_v|
    is symmetric): a host-side greedy rebalance assigns edges to windows so
    every window's global count is an exact multiple of 1024 (= 128 cols x 8
    cores), then deals each window's edges evenly across the 8 cores. The
    v-side gather stream is exactly 80000 indices per core - no padding.

v5 was v4 with the padding and overheads squeezed out:
  - Per-window edge counts are EXACT compile-time constants: the kernel is
    built after seeing the indices, so window w is padded only to
    round128(max over the 8 cores of count_w) instead of a uniform
    worst-case pad (92160 -> ~85k indices; the Q7 descriptor generator at
    ~8.5ns/idx is the bottleneck, so fewer indices = directly faster).
  - v-side dma_gather calls are batched over consecutive windows (up to
    7680 indices/call, ring-safe) to amortize per-call SWDGE overhead.
  - The one-hot is_equal compares use materialized iota tiles (both
    operands packed f16) so DVE runs them in 2x mode.
  - Everything else as v4: u-side gather via PE one-hot matmuls over
    256-node windows, d = |psum_u - g_v| (DVE subtract + i16 sign mask),
    mm1 packed halves, ACT relu, mm2 w2two-paired, ACT sigmoid.
"""

import os
import sys

for _p in ("/opt/trn_rl_repo", "/root/.axon_site/_ro/trn_rl_repo"):
    if os.path.isdir(_p) and _p not in sys.path:
        sys.path.insert(0, _p)

import numpy as np

import concourse.bacc as bacc
import concourse.mybir as mybir
from concourse.mybir import AluOpType
from concourse.tile import TileContext
from concourse.bass_utils import run_bass_kernel_spmd

N_NODES = 10000
N_EDGES = 640000
D_FEAT = 128
HID = 64
N_CORES = 8
E_CORE = N_EDGES // N_CORES      # 80000

W_WIN = 256
WINDOWS = 40                     # 40 * 256 = 10240 node-id range
N_PADNODES = WINDOWS * W_WIN
MAX_EPW = 2304                   # tile sizing bound for one window
BATCH_MAX = 2304                 # max gather indices per SWDGE call (per-window)

f16 = mybir.dt.float16
f32 = mybir.dt.float32
i16 = mybir.dt.int16

_NC_CACHE = {}


def _plan_batches(epws):
    """Group consecutive windows into gather batches of <= BATCH_MAX idxs."""
    batches = []
    cur = []
    tot = 0
    for w, n in enumerate(epws):
        if n == 0:
            continue
        if tot + n > BATCH_MAX and cur:
            batches.append(cur)
            cur, tot = [], 0
        cur.append(w)
        tot += n
    if cur:
        batches.append(cur)
    return batches


def _col_layout(epws):
    """Per-column padded-edge base, replicating the kernel's emission order."""
    base = []
    eoff_w = 0
    for n in epws:
        eoff = 0
        while eoff < n:
            na = min(512, n - eoff)
            nb = min(512, n - eoff - na)
            for j in range(na // 128):
                if j < nb // 128:
                    base.append(eoff_w + eoff + j * 128)
                    base.append(eoff_w + eoff + na + j * 128)
                else:
                    base.append(eoff_w + eoff + j * 128)
            eoff += na + nb
        eoff_w += n
    return np.asarray(base, dtype=np.int64)


def _build_nc(epws):
    e_pad = sum(epws)
    cols = e_pad // 128
    woff = np.concatenate([[0], np.cumsum(epws)]).astype(np.int64)

    nc = bacc.Bacc("TRN2", target_bir_lowering=False)

    x16_d = nc.dram_tensor("x16", [N_NODES, D_FEAT], f16, kind="ExternalInput")
    xpack_d = nc.dram_tensor("xpack", [128, N_PADNODES], f16,
                             kind="ExternalInput")
    idxv_d = nc.dram_tensor("idxv", [128, e_pad // 16], i16,
                            kind="ExternalInput")
    urel_d = nc.dram_tensor("urel", [1, e_pad], f16, kind="ExternalInput")
    iota_d = nc.dram_tensor("iotat", [128, 2 * MAX_EPW], f16,
                            kind="ExternalInput")
    w1_d = nc.dram_tensor("w1", [D_FEAT, HID], f16, kind="ExternalInput")
    w2two_d = nc.dram_tensor("w2two", [128, 2], f16, kind="ExternalInput")
    b1_d = nc.dram_tensor("b1", [128, 1], f32, kind="ExternalInput")
    b2_d = nc.dram_tensor("b2", [128, 1], f32, kind="ExternalInput")
    out_d = nc.dram_tensor("out", [128, cols], f32, kind="ExternalOutput")

    batches = _plan_batches(epws)

    with TileContext(nc) as tc:
        with (
            tc.tile_pool(name="const", bufs=1) as cpool,
            tc.tile_pool(name="gath", bufs=2) as gpool,
            tc.tile_pool(name="brc", bufs=3) as bpool,
            tc.tile_pool(name="cmp", bufs=3) as mpool,
            tc.tile_pool(name="diff", bufs=2) as dpool,
            tc.tile_pool(name="d2p", bufs=3) as d2pool,
            tc.tile_pool(name="hid", bufs=4) as hpool,
            tc.tile_pool(name="outp", bufs=1) as opool,
            tc.tile_pool(name="psu", bufs=3, space="PSUM") as upool,
            tc.tile_pool(name="ps1", bufs=3, space="PSUM") as ppool,
            tc.tile_pool(name="ps2", bufs=2, space="PSUM") as p2pool,
        ):
            xpack = cpool.tile([128, N_PADNODES], f16, tag="xpack")
            idxv = cpool.tile([128, e_pad // 16], i16, tag="idxv")
            iotat = cpool.tile([128, 2 * MAX_EPW], f16, tag="iotat")
            w1 = cpool.tile([D_FEAT, HID], f16, tag="w1")
            w2two = cpool.tile([128, 2], f16, tag="w2two")
            b1 = cpool.tile([128, 1], f32, tag="b1")
            b2 = cpool.tile([128, 1], f32, tag="b2")
            out_sb = opool.tile([128, cols], f32, tag="osb")

            nc.sync.dma_start(idxv[:], idxv_d[:])
            nc.sync.dma_start(xpack[:], xpack_d[:])
            nc.sync.dma_start(iotat[:], iota_d[:])
            nc.sync.dma_start(w1[:], w1_d[:])
            nc.sync.dma_start(w2two[:], w2two_d[:])
            nc.sync.dma_start(b1[:], b1_d[:])
            nc.sync.dma_start(b2[:], b2_d[:])

            col = 0
            flushed = 0
            for batch in batches:
                b0 = int(woff[batch[0]])
                blen = int(woff[batch[-1] + 1] - b0)
                gv = gpool.tile([128, BATCH_MAX], f16, tag="gv")
                nc.gpsimd.dma_gather(
                    gv[:, 0:blen].rearrange("p (a c) -> p a c", a=1),
                    x16_d[:], idxv[:, b0 // 16:(b0 + blen) // 16],
                    blen, blen,
                    elem_size=D_FEAT, transpose=True, single_packet=False,
                )
                bi = batches.index(batch)
                if bi in (10, 20, 30) and col > flushed:
                    nc.sync.dma_start(
                        out_d[:, flushed:col], out_sb[:, flushed:col])
                    flushed = col
                for w in batch:
                    n = epws[w]
                    e0 = int(woff[w])
                    g0 = e0 - b0  # offset within gv
                    ub = bpool.tile([128, MAX_EPW], f16, tag="ub")
                    nc.sync.dma_start(
                        ub[:, 0:n],
                        urel_d[:, e0:e0 + n].to_broadcast([128, n]),
                    )
                    cm0 = mpool.tile([128, MAX_EPW], f16, tag="cm0")
                    cm1 = mpool.tile([128, MAX_EPW], f16, tag="cm1")
                    nc.vector.tensor_tensor(
                        cm0[:, 0:n], ub[:, 0:n], iotat[:, 0:n],
                        AluOpType.is_equal)
                    nc.vector.tensor_tensor(
                        cm1[:, 0:n], ub[:, 0:n],
                        iotat[:, MAX_EPW:MAX_EPW + n],
                        AluOpType.is_equal)

                    blk0 = w * 2
                    d = dpool.tile([128, MAX_EPW], f16, tag="d")
                    for s0 in range(0, n, 512):
                        ns = min(512, n - s0)
                        pu = upool.tile([128, 512], f32, tag="pu")
                        nc.tensor.matmul(
                            pu[:, 0:ns],
                            xpack[:, blk0 * 128:(blk0 + 1) * 128],
                            cm0[:, s0:s0 + ns], start=True, stop=False,
                        )
                        nc.tensor.matmul(
                            pu[:, 0:ns],
                            xpack[:, (blk0 + 1) * 128:(blk0 + 2) * 128],
                            cm1[:, s0:s0 + ns], start=False, stop=True,
                        )
                        d2 = d2pool.tile([128, 512], f16, tag="d2")
                        nc.vector.tensor_tensor(
                            d[:, s0:s0 + ns], pu[:, 0:ns],
                            gv[:, g0 + s0:g0 + s0 + ns],
                            AluOpType.subtract)
                        nc.vector.tensor_tensor(
                            d2[:, 0:ns], gv[:, g0 + s0:g0 + s0 + ns],
                            pu[:, 0:ns], AluOpType.subtract)
                        nc.vector.tensor_tensor(
                            d[:, s0:s0 + ns], d[:, s0:s0 + ns], d2[:, 0:ns],
                            AluOpType.max)

                    eoff = 0
                    while eoff < n:
                        na = min(512, n - eoff)
                        nb = min(512, n - eoff - na)
                        pm = ppool.tile([128, 512], f32, tag="pm")
                        nc.tensor.matmul(
                            pm[0:HID, 0:na], w1[:], d[:, eoff:eoff + na],
                            start=True, stop=True,
                        )
                        if nb:
                            nc.tensor.matmul(
                                pm[HID:128, 0:nb], w1[:],
                                d[:, eoff + na:eoff + na + nb],
                                start=True, stop=True,
                            )
                        h = hpool.tile([128, 512], f16, tag="h")
                        if nb == na:
                            nc.scalar.activation(
                                h[:], pm[:],
                                mybir.ActivationFunctionType.Relu,
                                bias=b1[:], scale=1.0,
                            )
                        else:
                            nc.scalar.activation(
                                h[0:HID, 0:na], pm[0:HID, 0:na],
                                mybir.ActivationFunctionType.Relu,
                                bias=b1[0:HID, :], scale=1.0,
                            )
                        p2 = p2pool.tile([128, 8], f32, tag="p2")
                        np2 = 0
                        for j in range(na // 128):
                            if j < nb // 128:
                                nc.tensor.matmul(
                                    p2[:, np2:np2 + 2],
                                    h[:, j * 128:(j + 1) * 128], w2two[:],
                                    start=True, stop=True,
                                )
                                np2 += 2
                            else:
                                nc.tensor.matmul(
                                    p2[:, np2:np2 + 1],
                                    h[0:HID, j * 128:(j + 1) * 128],
                                    w2two[0:HID, 0:1],
                                    start=True, stop=True,
                                )
                                np2 += 1
                        nc.scalar.activation(
                            out_sb[:, col:col + np2], p2[:, 0:np2],
                            mybir.ActivationFunctionType.Sigmoid,
                            bias=b2[:], scale=1.0,
                        )
                        col += np2
                        eoff += na + nb

            assert col == cols
            nc.sync.dma_start(out_d[:, flushed:cols], out_sb[:, flushed:cols])

    nc.finalize()
    return nc


def _get_nc(epws):
    key = tuple(epws)
    if key not in _NC_CACHE:
        _NC_CACHE[key] = _build_nc(key)
    return _NC_CACHE[key]


def _plan_global(idx):
    """Assign each edge to the window of one endpoint so that every window's
    global count is a multiple of 1024, then deal evenly across cores.

    Returns (epws, per_core_plans) where per_core_plans[c] =
    (edges [e_core_pad] global edge ids, side [e_core_pad] 0=u-window,
     valid mask)."""
    u = np.asarray(idx[0], dtype=np.int64)
    v = np.asarray(idx[1], dtype=np.int64)
    wu = u // W_WIN
    wv = v // W_WIN
    side = np.zeros(N_EDGES, dtype=np.int8)
    cnt = np.bincount(wu, minlength=WINDOWS).astype(np.int64)

    # targets: multiples of 1024 summing to N_EDGES, tracking natural counts
    t = (cnt // 1024) * 1024
    rem = (N_EDGES - t.sum()) // 1024
    order = np.argsort(-(cnt - t))  # windows losing most get the spare units
    t[order[:rem]] += 1024
    assert t.sum() == N_EDGES and (t % 1024 == 0).all()

    # greedy repair: move flexible edges from over-full to under-full windows
    bywin = [list(np.nonzero(wu == w)[0]) for w in range(WINDOWS)]
    for _ in range(12):
        need = cnt - t
        if not need.any():
            break
        for w in range(WINDOWS):
            while need[w] > 0 and bywin[w]:
                moved = False
                for k in range(len(bywin[w]) - 1, -1, -1):
                    e = bywin[w][k]
                    tgt = wv[e]
                    if tgt != w and need[tgt] < 0:
                        bywin[w].pop(k)
                        side[e] = 1
                        need[w] -= 1
                        need[tgt] += 1
                        cnt[w] -= 1
                        cnt[tgt] += 1
                        moved = True
                        if need[w] <= 0:
                            break
                if not moved:
                    break
    assert (cnt == t).all(), f"rebalance failed: {cnt - t}"

    # collect per-window edge lists (u-side edges + moved v-side edges)
    win_of = np.where(side == 0, wu, wv)
    epws = tuple(int(x) // N_CORES for x in t)  # per-core window sizes
    e_core = sum(epws)
    assert e_core == N_EDGES // N_CORES

    plans = []
    ordglob = np.argsort(win_of, kind="stable")
    starts = np.searchsorted(win_of[ordglob], np.arange(WINDOWS + 1))
    percore_edges = np.empty((N_CORES, e_core), dtype=np.int64)
    woff = np.concatenate([[0], np.cumsum(epws)]).astype(np.int64)
    for w in range(WINDOWS):
        es = ordglob[starts[w]:starts[w + 1]]
        n = epws[w]
        for c in range(N_CORES):
            percore_edges[c, woff[w]:woff[w] + n] = es[c * n:(c + 1) * n]
    for c in range(N_CORES):
        plans.append((percore_edges[c], side[percore_edges[c]]))
    return epws, plans


def prep_in_maps(x, indices, W1, b1, W2, b2, epws, plans):
    x16 = np.asarray(x, dtype=np.float32).astype(np.float16)
    idx = np.asarray(indices)
    u = np.asarray(idx[0], dtype=np.int64)
    v = np.asarray(idx[1], dtype=np.int64)
    e_pad = sum(epws)
    woff = np.concatenate([[0], np.cumsum(epws)]).astype(np.int64)
    xpack = np.zeros((128, N_PADNODES), dtype=np.float16)
    for r in range(N_PADNODES // 128):
        n0, n1 = r * 128, min(r * 128 + 128, N_NODES)
        if n1 > n0:
            xpack[: n1 - n0, r * 128:r * 128 + 128] = x16[n0:n1]
    iotat = np.zeros((128, 2 * MAX_EPW), dtype=np.float16)
    iotat[:, 0:MAX_EPW] = np.arange(128, dtype=np.float16)[:, None]
    iotat[:, MAX_EPW:] = (np.arange(128) + 128).astype(np.float16)[:, None]

    w1 = np.asarray(W1, dtype=np.float32).astype(np.float16)
    w2c = np.asarray(W2, dtype=np.float32).astype(np.float16).reshape(HID)
    w2two = np.zeros((128, 2), dtype=np.float16)
    w2two[0:HID, 0] = w2c
    w2two[HID:128, 1] = w2c
    b1c = np.asarray(b1, dtype=np.float32).reshape(HID, 1)
    b1s = np.concatenate([b1c, b1c], axis=0)
    b2s = np.full((128, 1), np.asarray(b2, dtype=np.float32).reshape(-1)[0],
                  dtype=np.float32)

    in_maps = []
    for c in range(N_CORES):
        edges, sd = plans[c]
        # PE-side node (one-hot) and Q7-side node (dma_gather)
        pe_node = np.where(sd == 0, u[edges], v[edges])
        q7_node = np.where(sd == 0, v[edges], u[edges])
        urel = np.zeros(e_pad, dtype=np.float16)
        for w in range(WINDOWS):
            s, e = int(woff[w]), int(woff[w + 1])
            urel[s:e] = (pe_node[s:e] - w * W_WIN).astype(np.float16)
        vsel = q7_node.astype(np.int16)
        idxv = np.tile(vsel.reshape(-1, 16).T, (8, 1)).astype(np.int16)
        in_maps.append({
            "x16": x16, "xpack": xpack,
            "idxv": idxv, "urel": urel.reshape(1, e_pad),
            "iotat": iotat, "w1": w1, "w2two": w2two,
            "b1": b1s, "b2": b2s,
        })
    return in_maps


def run_hw(x, indices, W1, b1, W2, b2, trace=False, **kw):
    epws, plans = _plan_global(np.asarray(indices))
    nc = _get_nc(epws)
    in_maps = prep_in_maps(x, indices, W1, b1, W2, b2, epws, plans)
    res = run_bass_kernel_spmd(
        nc, in_maps, core_ids=list(range(N_CORES)), trace=trace, **kw
    )
    base = _col_layout(epws)
    pedge = base[None, :] + np.arange(128)[:, None]
    e_pad = sum(epws)
    out = np.empty(N_EDGES, dtype=np.float32)
    for c in range(N_CORES):
        o = np.asarray(res.results[c]["out"])
        edges, _sd = plans[c]
        flat = np.empty(e_pad, dtype=np.float32)
        flat[pedge.ravel()] = o.ravel()
        out[edges] = flat
    return out, res


def kernel(x, indices, W1, b1, W2, b2):
    out, _ = run_hw(x, indices, W1, b1, W2, b2, trace=False)
    return out.astype(np.float32)
